# revision 12
# baseline (speedup 1.0000x reference)
"""Trainium2 Bass kernel for the 3-layer LSTM scan (nn_Net_2095944040841).

Architecture (per core, batch-sharded 512/8 = 64):
  reference: hid = x @ W1.T + b1; 3 chained LSTMCells over T=1024 with the
  original model's quirky state handling (c3 stays 0; cell3 overwrites c2,
  which cell2 reads the next step).

Mapping (v3):
  - Gate rows on partitions packed [i; f] and [g; o]; batch on the free axis.
  - K-stacked stationaries: each (cell, gate-group) is ONE matmul with a
    [128, 128] stationary holding input weights (rows 0:K_in), the bias
    (row 63) and recurrent weights (rows 64:114). The moving state tile per
    group/parity holds [input | bias | h_self] on the matching rows, so a
    tick needs only 6 gate matmuls per group.
  - Both gate groups of all 3 cells share ONE PSUM tile so a single Tanh
    covers every gate per tick.
  - Sigmoids via tanh (weights pre-scaled); states scaled h_hat=2h, s_hat=2c.
  - Skew: at tick k cell1 computes t=k, cell2 t=k-1, cell3 t=k-2. Blocks
    [c1|c2|c3]. Cell3's c-input is 0; cell2's c-input is cell3's same-tick
    i*g (a2 blk3 stays 0 so one stt updates every c).
  - h is written twice by two parallel stts: into self rows (64:128) and into
    the next cells' input rows (0:50) - no serial staging copy.
  - Output matmul batched over OB ticks from an h3 history buffer.
  - Two batch groups emitted 4 pipeline stages out of phase to hide the
    serial chain latency (engines execute queues in order).
  - Small elementwise ops (a2, x/hist copies) run on GPSIMD.
"""

import sys

sys.path.insert(0, "/opt/trn_rl_repo")

import numpy as np

import concourse.bass as bass
import concourse.tile as tile
from concourse import bacc, mybir

HID = 50
IN_DIM = 20
OUT_DIM = 8
T_FULL = 1024
N_CORES = 8

b = 64          # batch per core
G = 2           # interleaved batch groups per core
bg = b // G     # batch per group
TB = 3 * bg     # merged tile width per group; blocks [c1 | c2 | c3]
CH = 64         # x-chunk length in ticks
OB = 8          # output accumulation ticks per DMA
BIAS_ROW = 63   # constant-1.0 row in the moving state tile
NS = 8          # pipeline stages per tick
PHASE = 4       # group-1 lag in stages

F32 = mybir.dt.float32
BF16 = mybir.dt.bfloat16
CDT = BF16
import ml_dtypes
NP_CDT = ml_dtypes.bfloat16

GATES = {"i": slice(0, 50), "f": slice(50, 100), "g": slice(100, 150),
         "o": slice(150, 200)}


def prep_params(W1, b1, Wih1, Whh1, bih1, bhh1, Wih2, Whh2, bih2, bhh2,
                Wih3, Whh3, bih3, bhh3, W2, b2):
    """Host-side weight transformation -> {name: np.float32 array}.

    Stationary per (cell, group) is [128, 128]:
      rows 0:K_in   = input weights (cell1: Wih1@W1 on x; cells2/3: Wih on
                      h_hat_prev, pre-halved)
      row  BIAS_ROW = total bias
      rows 64:114   = recurrent weights on h_hat_self (pre-halved)
    columns: gate A -> 0:50, gate B -> 64:114 (groups "if", "go").
    """
    W1 = np.asarray(W1, np.float32)
    Wc1 = np.asarray(Wih1, np.float32) @ W1            # [200, 20]
    bc1 = (np.asarray(Wih1, np.float32) @ np.asarray(b1, np.float32)
           + np.asarray(bih1, np.float32) + np.asarray(bhh1, np.float32))
    cells = {
        1: (Wc1, np.asarray(Whh1, np.float32), bc1, 1.0),
        2: (np.asarray(Wih2, np.float32), np.asarray(Whh2, np.float32),
            np.asarray(bih2, np.float32) + np.asarray(bhh2, np.float32), 0.5),
        3: (np.asarray(Wih3, np.float32), np.asarray(Whh3, np.float32),
            np.asarray(bih3, np.float32) + np.asarray(bhh3, np.float32), 0.5),
    }
    out = {}
    for c, (Wx, Wh, bias, in_scale) in cells.items():
        gs = {g: (0.5 if g in "ifo" else 1.0) for g in "ifgo"}
        blk = {g: gs[g] * in_scale * Wx[GATES[g]] for g in "ifgo"}   # [50, K]
        blkh = {g: gs[g] * 0.5 * Wh[GATES[g]] for g in "ifgo"}       # [50, 50]
        bb = {g: gs[g] * bias[GATES[g]] for g in "ifgo"}
        Kx = Wx.shape[1]
        for gname, (ga, gb) in (("if", ("i", "f")), ("go", ("g", "o"))):
            w = np.zeros((128, 128), np.float32)
            w[0:Kx, 0:50] = blk[ga].T
            w[0:Kx, 64:114] = blk[gb].T
            w[BIAS_ROW, 0:50] = bb[ga]
            w[BIAS_ROW, 64:114] = bb[gb]
            w[64:114, 0:50] = blkh[ga].T
            w[64:114, 64:114] = blkh[gb].T
            out[f"w{c}_{gname}"] = w
    w2e = np.zeros((51, OUT_DIM), np.float32)
    w2e[0:50, :] = 0.5 * np.asarray(W2, np.float32).T
    w2e[50, :] = np.asarray(b2, np.float32)
    out["w2e"] = w2e
    return out


def build_nc(T=T_FULL):
    """Build the Bass module for one core (SPMD across 8)."""
    nc = bacc.Bacc(None, target_bir_lowering=False)
    BLK = [0, bg, 2 * bg]

    xt = nc.dram_tensor("xt", [IN_DIM, T, b], CDT, kind="ExternalInput")
    wnames = {}
    for c in (1, 2, 3):
        for g in ("if", "go"):
            wnames[f"w{c}_{g}"] = nc.dram_tensor(
                f"w{c}_{g}", [128, 128], CDT, kind="ExternalInput")
    w2e_d = nc.dram_tensor("w2e", [51, OUT_DIM], CDT, kind="ExternalInput")
    ones_d = nc.dram_tensor("ones", [1, OB * b], CDT, kind="ExternalInput")
    out_d = nc.dram_tensor("out", [T, OUT_DIM, b], F32, kind="ExternalOutput")

    n_chunks = (T + CH - 1) // CH
    Tanh = mybir.ActivationFunctionType.Tanh
    ADD, MULT = mybir.AluOpType.add, mybir.AluOpType.mult

    with tile.TileContext(nc) as tc:
        with (
            tc.tile_pool(name="weights", bufs=1) as wp,
            tc.tile_pool(name="state", bufs=1) as sp,
            tc.tile_pool(name="xs", bufs=1) as xp,
            tc.tile_pool(name="work", bufs=3) as wk,
            tc.tile_pool(name="psum", bufs=2, space="PSUM") as pp,
            tc.tile_pool(name="opsum", bufs=1, space="PSUM") as op_pool,
        ):
            wt = {}
            for name, d in wnames.items():
                t = wp.tile(list(d.shape), CDT, name=name, tag=name)
                nc.sync.dma_start(t[:], d[:])
                wt[name] = t
            w2e = wp.tile([128, OUT_DIM], CDT)
            nc.sync.dma_start(w2e[64:115, :], w2e_d[:])

            # moving state tiles: rows 0:50 input (x / h_hat_prev),
            # row 63 = 1.0, rows 64:128 h_hat_self (+junk), per group/parity
            M = [[sp.tile([128, TB], CDT, name=f"M{g}_{i}", tag=f"M{g}_{i}")
                  for i in range(2)] for g in range(G)]
            s_ring = [[sp.tile([128, TB], CDT, name=f"s{g}_{i}", tag=f"s{g}_{i}")
                       for i in range(2)] for g in range(G)]
            a2t = [[sp.tile([128, TB], CDT, name=f"a2_{g}_{i}", tag=f"a2_{g}_{i}")
                    for i in range(2)] for g in range(G)]
            hist = [sp.tile([128, OB * b], CDT, name=f"hist{i}", tag=f"hist{i}")
                    for i in range(2)]
            for g in range(G):
                for i in range(2):
                    nc.vector.memset(M[g][i][0:64, :], 0.0)
                    nc.vector.memset(M[g][i][64:128, :], 0.0)
                    nc.sync.dma_start(M[g][i][BIAS_ROW:BIAS_ROW + 1, :],
                                      ones_d[:, 0:TB])
                    nc.vector.memset(s_ring[g][i][64:128, :], 0.0)
                    nc.vector.memset(a2t[g][i][64:128, 2 * bg:3 * bg], 0.0)
            for i in range(2):
                nc.vector.memset(hist[i][64:128, :], 0.0)
                nc.sync.dma_start(hist[i][114:115, :], ones_d[:])

            xs_ring = [xp.tile([IN_DIM, CH, b], CDT, name=f"xs{i}", tag=f"xs{i}")
                       for i in range(2)]
            nc.sync.dma_start(xs_ring[0][:], xt[:, 0:CH, :])
            # pre-stage x(0) into the tick-0 moving tiles (parity 1)
            for g in range(G):
                nc.gpsimd.tensor_copy(
                    M[g][1][0:IN_DIM, 0:bg],
                    xs_ring[0][:, 0, g * bg:(g + 1) * bg])

            out_ring = [op_pool.tile([OUT_DIM, OB * b], F32,
                                     name=f"ob{i}", tag=f"ob{i}")
                        for i in range(2)]

            # per-(group, parity) work tiles are pooled by tag
            cur = {}   # live tiles per group: P, S, at1, tcx

            def stage(g, k, s):
                p, q = k % 2, (k - 1) % 2
                t1 = min(k, T - 1)
                c_idx = t1 // CH
                if s == 0:
                    if g == 0 and k % CH == 0 and k // CH == c_idx \
                            and c_idx + 1 < n_chunks:
                        nc.sync.dma_start(
                            xs_ring[(c_idx + 1) % 2][:],
                            xt[:, (c_idx + 1) * CH:(c_idx + 2) * CH, :])
                    P = pp.tile([128, 2 * TB], F32, name=f"P{g}", tag=f"P{g}")
                    cur[(g, "P")] = P
                    for (gg, off) in (("if", 0), ("go", TB)):
                        for c in (1, 2, 3):
                            lo = off + BLK[c - 1]
                            nc.tensor.matmul(
                                P[:, lo:lo + bg], wt[f"w{c}_{gg}"][:],
                                M[g][q][:, BLK[c - 1]:BLK[c - 1] + bg],
                                start=True, stop=True)
                elif s == 1:
                    S = wk.tile([128, 2 * TB], CDT, name=f"S{g}", tag=f"S{g}")
                    cur[(g, "S")] = S
                    nc.scalar.activation(S[:], cur[(g, "P")][:], Tanh)
                elif s == 2:
                    S = cur[(g, "S")]
                    at1 = wk.tile([128, TB], CDT, name=f"at1_{g}",
                                  tag=f"at1_{g}")
                    cur[(g, "at1")] = at1
                    nc.vector.scalar_tensor_tensor(
                        at1[64:128, :], S[0:64, 0:TB], 1.0, S[0:64, TB:2 * TB],
                        ADD, MULT)
                elif s == 3:
                    S, at1 = cur[(g, "S")], cur[(g, "at1")]
                    a2 = a2t[g][p]
                    nc.vector.scalar_tensor_tensor(
                        a2[64:128, 0:bg], S[64:128, 0:bg], 1.0,
                        s_ring[g][q][64:128, 0:bg], ADD, MULT)
                    if k == 1:
                        nc.vector.memset(a2[64:128, bg:2 * bg], 0.0)
                    else:
                        nc.vector.scalar_tensor_tensor(
                            a2[64:128, bg:2 * bg], S[64:128, bg:2 * bg], 1.0,
                            at1[64:128, 2 * bg:3 * bg], ADD, MULT)
                elif s == 4:
                    nc.vector.scalar_tensor_tensor(
                        s_ring[g][p][64:128, :], a2t[g][p][64:128, :], 0.5,
                        cur[(g, "at1")][64:128, :], MULT, ADD)
                elif s == 5:
                    tcx = wk.tile([128, TB], CDT, name=f"tc{g}", tag=f"tc{g}")
                    cur[(g, "tcx")] = tcx
                    nc.scalar.activation(tcx[64:128, :],
                                         s_ring[g][p][64:128, :], Tanh,
                                         scale=0.5)
                elif s == 6:
                    S, tcx = cur[(g, "S")], cur[(g, "tcx")]
                    # h_hat into self rows of M[p] (junk rows 114:128 hit
                    # zero stationary rows)
                    nc.vector.scalar_tensor_tensor(
                        M[g][p][64:128, :], S[64:128, TB:2 * TB], 1.0,
                        tcx[64:128, :], ADD, MULT)
                    # h_hat of cells 1,2 into input rows of blocks 2,3
                    nc.gpsimd.tensor_copy(M[g][p][0:50, bg:3 * bg],
                                          M[g][p][64:114, 0:2 * bg])
                    if k == 0:
                        # cells 2,3 computed garbage steps t<0: zero their
                        # self-h; zero cell3's input h2 (t=-1). Cell2's input
                        # h1(t=0) is real and must survive.
                        nc.vector.memset(M[g][0][64:128, bg:3 * bg], 0.0)
                        nc.vector.memset(M[g][0][0:50, 2 * bg:3 * bg], 0.0)
                    elif k == 1:
                        # cell3 is still at t=-1: zero only its self-h
                        nc.vector.memset(M[g][1][64:128, 2 * bg:3 * bg], 0.0)
                    # x(t=k+1) into input rows of block 1 (next parity tile)
                    tn = min(k + 1, T - 1)
                    nc.gpsimd.tensor_copy(
                        M[g][p][0:IN_DIM, 0:bg],
                        xs_ring[(tn // CH) % 2][:, tn % CH,
                                                g * bg:(g + 1) * bg])
                    # h3_hat into the history buffer for the batched out-mm
                    if k >= 2:
                        t3 = k - 2
                        nc.gpsimd.tensor_copy(
                            hist[(t3 // OB) % 2][64:114,
                                                 (t3 % OB) * b + g * bg:
                                                 (t3 % OB) * b + (g + 1) * bg],
                            M[g][p][64:114, 2 * bg:3 * bg])
                elif s == 7:
                    # trailing group emits the batched output for both groups
                    if g != G - 1 or k < 2:
                        return
                    t3 = k - 2
                    if t3 % OB != OB - 1:
                        return
                    oslot = (t3 // OB) % 2
                    t0 = t3 - OB + 1
                    nc.tensor.matmul(out_ring[oslot][:],
                                     w2e[64:115, :], hist[oslot][64:115, :],
                                     start=True, stop=True)
                    ob_sb = wk.tile([OUT_DIM, OB * b], F32, name="ob_sb",
                                    tag="ob_sb")
                    nc.scalar.copy(ob_sb[:], out_ring[oslot][:])
                    nc.sync.dma_start(
                        out_d[t0:t0 + OB, :, :].rearrange("t p c -> p t c"),
                        ob_sb[:].rearrange("p (t c) -> p t c", t=OB))

            total = NS * (T + 2)
            for tau in range(total + PHASE):
                if tau < total:
                    stage(0, tau // NS, tau % NS)
                t2 = tau - PHASE
                if 0 <= t2 < total:
                    stage(1, t2 // NS, t2 % NS)
    nc.compile()
    return nc


def make_in_maps(inputs):
    x = np.asarray(inputs["x"], np.float32)          # [512, 1024, 20]
    params = prep_params(**{k: v for k, v in inputs.items() if k != "x"})
    in_maps = []
    for c in range(N_CORES):
        xc = x[c * b:(c + 1) * b]                    # [64, T, 20]
        xtc = np.ascontiguousarray(xc.transpose(2, 1, 0))   # [20, T, 64]
        m = {"xt": xtc.astype(NP_CDT),
             "ones": np.ones((1, OB * b), NP_CDT)}
        m.update({k: v.astype(NP_CDT) for k, v in params.items()})
        in_maps.append(m)
    return in_maps


def gather_out(res, B, T):
    out = np.empty((B, T, OUT_DIM), np.float32)
    for c in range(N_CORES):
        out[c * b:(c + 1) * b] = res.results[c]["out"].transpose(2, 0, 1)
    return out


def kernel(**inputs):
    from concourse.bass_utils import run_bass_kernel_spmd

    x = np.asarray(inputs["x"], np.float32)
    B, T, _ = x.shape
    nc = build_nc(T)
    in_maps = make_in_maps(inputs)

    res = run_bass_kernel_spmd(nc, in_maps, core_ids=list(range(N_CORES)))
    return gather_out(res, B, T)


# revision 13
# speedup vs baseline: 1.1759x; 1.1759x over previous
"""Trainium2 Bass kernel for the 3-layer LSTM scan (nn_Net_2095944040841).

Architecture (per core, batch-sharded 512/8 = 64):
  reference: hid = x @ W1.T + b1; 3 chained LSTMCells over T=1024 with the
  original model's quirky state handling (c3 stays 0; cell3 overwrites c2,
  which cell2 reads the next step).

Mapping (v3):
  - Gate rows on partitions packed [i; f] and [g; o]; batch on the free axis.
  - K-stacked stationaries: each (cell, gate-group) is ONE matmul with a
    [128, 128] stationary holding input weights (rows 0:K_in), the bias
    (row 63) and recurrent weights (rows 64:114). The moving state tile per
    group/parity holds [input | bias | h_self] on the matching rows, so a
    tick needs only 6 gate matmuls per group.
  - Both gate groups of all 3 cells share ONE PSUM tile so a single Tanh
    covers every gate per tick.
  - Sigmoids via tanh (weights pre-scaled); states scaled h_hat=2h, s_hat=2c.
  - Skew: at tick k cell1 computes t=k, cell2 t=k-1, cell3 t=k-2. Blocks
    [c1|c2|c3]. Cell3's c-input is 0; cell2's c-input is cell3's same-tick
    i*g (a2 blk3 stays 0 so one stt updates every c).
  - h is written twice by two parallel stts: into self rows (64:128) and into
    the next cells' input rows (0:50) - no serial staging copy.
  - Output matmul batched over OB ticks from an h3 history buffer.
  - Two batch groups emitted 4 pipeline stages out of phase to hide the
    serial chain latency (engines execute queues in order).
  - Small elementwise ops (a2, x/hist copies) run on GPSIMD.
"""

import sys

sys.path.insert(0, "/opt/trn_rl_repo")

import numpy as np

import concourse.bass as bass
import concourse.tile as tile
from concourse import bacc, mybir

HID = 50
IN_DIM = 20
OUT_DIM = 8
T_FULL = 1024
N_CORES = 8

b = 64          # batch per core
G = 2           # interleaved batch groups per core
bg = b // G     # batch per group
TB = 3 * bg     # merged tile width per group; blocks [c1 | c2 | c3]
CH = 64         # x-chunk length in ticks
OB = 8          # output accumulation ticks per DMA
BIAS_ROW = 63   # constant-1.0 row in the moving state tile
NS = 8          # pipeline stages per tick
PHASE = 3       # group-1 lag in stages

F32 = mybir.dt.float32
BF16 = mybir.dt.bfloat16
CDT = BF16
import ml_dtypes
NP_CDT = ml_dtypes.bfloat16

GATES = {"i": slice(0, 50), "f": slice(50, 100), "g": slice(100, 150),
         "o": slice(150, 200)}


def prep_params(W1, b1, Wih1, Whh1, bih1, bhh1, Wih2, Whh2, bih2, bhh2,
                Wih3, Whh3, bih3, bhh3, W2, b2):
    """Host-side weight transformation -> {name: np.float32 array}.

    Stationary per (cell, group) is [128, 128]:
      rows 0:K_in   = input weights (cell1: Wih1@W1 on x; cells2/3: Wih on
                      h_hat_prev, pre-halved)
      row  BIAS_ROW = total bias
      rows 64:114   = recurrent weights on h_hat_self (pre-halved)
    columns: gate A -> 0:50, gate B -> 64:114 (groups "if", "go").
    """
    W1 = np.asarray(W1, np.float32)
    Wc1 = np.asarray(Wih1, np.float32) @ W1            # [200, 20]
    bc1 = (np.asarray(Wih1, np.float32) @ np.asarray(b1, np.float32)
           + np.asarray(bih1, np.float32) + np.asarray(bhh1, np.float32))
    cells = {
        1: (Wc1, np.asarray(Whh1, np.float32), bc1, 1.0),
        2: (np.asarray(Wih2, np.float32), np.asarray(Whh2, np.float32),
            np.asarray(bih2, np.float32) + np.asarray(bhh2, np.float32), 0.5),
        3: (np.asarray(Wih3, np.float32), np.asarray(Whh3, np.float32),
            np.asarray(bih3, np.float32) + np.asarray(bhh3, np.float32), 0.5),
    }
    out = {}
    for c, (Wx, Wh, bias, in_scale) in cells.items():
        gs = {g: (0.5 if g in "ifo" else 1.0) for g in "ifgo"}
        blk = {g: gs[g] * in_scale * Wx[GATES[g]] for g in "ifgo"}   # [50, K]
        blkh = {g: gs[g] * 0.5 * Wh[GATES[g]] for g in "ifgo"}       # [50, 50]
        bb = {g: gs[g] * bias[GATES[g]] for g in "ifgo"}
        Kx = Wx.shape[1]
        for gname, (ga, gb) in (("if", ("i", "f")), ("go", ("g", "o"))):
            w = np.zeros((128, 128), np.float32)
            w[0:Kx, 0:50] = blk[ga].T
            w[0:Kx, 64:114] = blk[gb].T
            w[BIAS_ROW, 0:50] = bb[ga]
            w[BIAS_ROW, 64:114] = bb[gb]
            w[64:114, 0:50] = blkh[ga].T
            w[64:114, 64:114] = blkh[gb].T
            out[f"w{c}_{gname}"] = w
    w2e = np.zeros((51, OUT_DIM), np.float32)
    w2e[0:50, :] = 0.5 * np.asarray(W2, np.float32).T
    w2e[50, :] = np.asarray(b2, np.float32)
    out["w2e"] = w2e
    return out


def build_nc(T=T_FULL):
    """Build the Bass module for one core (SPMD across 8)."""
    nc = bacc.Bacc(None, target_bir_lowering=False)
    BLK = [0, bg, 2 * bg]

    xt = nc.dram_tensor("xt", [IN_DIM, T, b], CDT, kind="ExternalInput")
    wnames = {}
    for c in (1, 2, 3):
        for g in ("if", "go"):
            wnames[f"w{c}_{g}"] = nc.dram_tensor(
                f"w{c}_{g}", [128, 128], CDT, kind="ExternalInput")
    w2e_d = nc.dram_tensor("w2e", [51, OUT_DIM], CDT, kind="ExternalInput")
    ones_d = nc.dram_tensor("ones", [1, OB * b], CDT, kind="ExternalInput")
    out_d = nc.dram_tensor("out", [T, OUT_DIM, b], F32, kind="ExternalOutput")

    n_chunks = (T + CH - 1) // CH
    Tanh = mybir.ActivationFunctionType.Tanh
    ADD, MULT = mybir.AluOpType.add, mybir.AluOpType.mult

    with tile.TileContext(nc) as tc:
        with (
            tc.tile_pool(name="weights", bufs=1) as wp,
            tc.tile_pool(name="state", bufs=1) as sp,
            tc.tile_pool(name="xs", bufs=1) as xp,
            tc.tile_pool(name="work", bufs=3) as wk,
            tc.tile_pool(name="psum", bufs=2, space="PSUM") as pp,
            tc.tile_pool(name="opsum", bufs=1, space="PSUM") as op_pool,
        ):
            wt = {}
            for name, d in wnames.items():
                t = wp.tile(list(d.shape), CDT, name=name, tag=name)
                nc.sync.dma_start(t[:], d[:])
                wt[name] = t
            w2e = wp.tile([128, OUT_DIM], CDT)
            nc.sync.dma_start(w2e[64:115, :], w2e_d[:])

            # moving state tiles: rows 0:50 input (x / h_hat_prev),
            # row 63 = 1.0, rows 64:128 h_hat_self (+junk), per group/parity
            M = [[sp.tile([128, TB], CDT, name=f"M{g}_{i}", tag=f"M{g}_{i}")
                  for i in range(2)] for g in range(G)]
            s_ring = [[sp.tile([128, TB], CDT, name=f"s{g}_{i}", tag=f"s{g}_{i}")
                       for i in range(2)] for g in range(G)]
            a2t = [[sp.tile([128, TB], CDT, name=f"a2_{g}_{i}", tag=f"a2_{g}_{i}")
                    for i in range(2)] for g in range(G)]
            hist = [sp.tile([128, OB * b], CDT, name=f"hist{i}", tag=f"hist{i}")
                    for i in range(2)]
            for g in range(G):
                for i in range(2):
                    nc.vector.memset(M[g][i][0:64, :], 0.0)
                    nc.vector.memset(M[g][i][64:128, :], 0.0)
                    nc.sync.dma_start(M[g][i][BIAS_ROW:BIAS_ROW + 1, :],
                                      ones_d[:, 0:TB])
                    nc.vector.memset(s_ring[g][i][64:128, :], 0.0)
                    nc.vector.memset(a2t[g][i][64:128, 2 * bg:3 * bg], 0.0)
            for i in range(2):
                nc.vector.memset(hist[i][64:128, :], 0.0)
                nc.sync.dma_start(hist[i][114:115, :], ones_d[:])

            xs_ring = [xp.tile([IN_DIM, CH, b], CDT, name=f"xs{i}", tag=f"xs{i}")
                       for i in range(2)]
            nc.sync.dma_start(xs_ring[0][:], xt[:, 0:CH, :])
            # pre-stage x(0) into the tick-0 moving tiles (parity 1)
            for g in range(G):
                nc.gpsimd.tensor_copy(
                    M[g][1][0:IN_DIM, 0:bg],
                    xs_ring[0][:, 0, g * bg:(g + 1) * bg])

            out_ring = [op_pool.tile([OUT_DIM, OB * b], F32,
                                     name=f"ob{i}", tag=f"ob{i}")
                        for i in range(2)]

            # per-(group, parity) work tiles are pooled by tag
            cur = {}   # live tiles per group: P, S, at1, tcx

            def stage(g, k, s):
                p, q = k % 2, (k - 1) % 2
                t1 = min(k, T - 1)
                c_idx = t1 // CH
                if s == 0:
                    if g == 0 and k % CH == 0 and k // CH == c_idx \
                            and c_idx + 1 < n_chunks:
                        nc.sync.dma_start(
                            xs_ring[(c_idx + 1) % 2][:],
                            xt[:, (c_idx + 1) * CH:(c_idx + 2) * CH, :])
                    P = pp.tile([128, 2 * TB], F32, name=f"P{g}", tag=f"P{g}")
                    cur[(g, "P")] = P
                    for (gg, off) in (("if", 0), ("go", TB)):
                        for c in (1, 2, 3):
                            lo = off + BLK[c - 1]
                            nc.tensor.matmul(
                                P[:, lo:lo + bg], wt[f"w{c}_{gg}"][:],
                                M[g][q][:, BLK[c - 1]:BLK[c - 1] + bg],
                                start=True, stop=True)
                elif s == 1:
                    S = wk.tile([128, 2 * TB], CDT, name=f"S{g}", tag=f"S{g}")
                    cur[(g, "S")] = S
                    nc.scalar.activation(S[:], cur[(g, "P")][:], Tanh)
                elif s == 2:
                    S = cur[(g, "S")]
                    at1 = wk.tile([128, TB], CDT, name=f"at1_{g}",
                                  tag=f"at1_{g}")
                    cur[(g, "at1")] = at1
                    nc.vector.scalar_tensor_tensor(
                        at1[64:128, :], S[0:64, 0:TB], 1.0, S[0:64, TB:2 * TB],
                        ADD, MULT)
                elif s == 3:
                    S, at1 = cur[(g, "S")], cur[(g, "at1")]
                    a2 = a2t[g][p]
                    nc.vector.scalar_tensor_tensor(
                        a2[64:128, 0:bg], S[64:128, 0:bg], 1.0,
                        s_ring[g][q][64:128, 0:bg], ADD, MULT)
                    if k == 1:
                        nc.vector.memset(a2[64:128, bg:2 * bg], 0.0)
                    else:
                        nc.vector.scalar_tensor_tensor(
                            a2[64:128, bg:2 * bg], S[64:128, bg:2 * bg], 1.0,
                            at1[64:128, 2 * bg:3 * bg], ADD, MULT)
                elif s == 4:
                    nc.vector.scalar_tensor_tensor(
                        s_ring[g][p][64:128, :], a2t[g][p][64:128, :], 0.5,
                        cur[(g, "at1")][64:128, :], MULT, ADD)
                elif s == 5:
                    tcx = wk.tile([128, TB], CDT, name=f"tc{g}", tag=f"tc{g}")
                    cur[(g, "tcx")] = tcx
                    nc.scalar.activation(tcx[64:128, :],
                                         s_ring[g][p][64:128, :], Tanh,
                                         scale=0.5)
                elif s == 6:
                    S, tcx = cur[(g, "S")], cur[(g, "tcx")]
                    # h_hat into self rows of M[p] (junk rows 114:128 hit
                    # zero stationary rows)
                    nc.vector.scalar_tensor_tensor(
                        M[g][p][64:128, :], S[64:128, TB:2 * TB], 1.0,
                        tcx[64:128, :], ADD, MULT)
                    # h_hat of cells 1,2 into input rows of blocks 2,3
                    nc.vector.scalar_tensor_tensor(
                        M[g][p][0:50, bg:3 * bg],
                        S[64:114, TB:TB + 2 * bg], 1.0,
                        tcx[64:114, 0:2 * bg], ADD, MULT)
                    if k == 0:
                        # cells 2,3 computed garbage steps t<0: zero their
                        # self-h; zero cell3's input h2 (t=-1). Cell2's input
                        # h1(t=0) is real and must survive.
                        nc.vector.memset(M[g][0][64:128, bg:3 * bg], 0.0)
                        nc.vector.memset(M[g][0][0:50, 2 * bg:3 * bg], 0.0)
                    elif k == 1:
                        # cell3 is still at t=-1: zero only its self-h
                        nc.vector.memset(M[g][1][64:128, 2 * bg:3 * bg], 0.0)
                    # x(t=k+1) into input rows of block 1 (next parity tile)
                    tn = min(k + 1, T - 1)
                    nc.gpsimd.tensor_copy(
                        M[g][p][0:IN_DIM, 0:bg],
                        xs_ring[(tn // CH) % 2][:, tn % CH,
                                                g * bg:(g + 1) * bg])
                    # h3_hat into the history buffer for the batched out-mm
                    if k >= 2:
                        t3 = k - 2
                        nc.gpsimd.tensor_copy(
                            hist[(t3 // OB) % 2][64:114,
                                                 (t3 % OB) * b + g * bg:
                                                 (t3 % OB) * b + (g + 1) * bg],
                            M[g][p][64:114, 2 * bg:3 * bg])
                elif s == 7:
                    # trailing group emits the batched output for both groups
                    if g != G - 1 or k < 2:
                        return
                    t3 = k - 2
                    if t3 % OB != OB - 1:
                        return
                    oslot = (t3 // OB) % 2
                    t0 = t3 - OB + 1
                    nc.tensor.matmul(out_ring[oslot][:],
                                     w2e[64:115, :], hist[oslot][64:115, :],
                                     start=True, stop=True)
                    ob_sb = wk.tile([OUT_DIM, OB * b], F32, name="ob_sb",
                                    tag="ob_sb")
                    nc.scalar.copy(ob_sb[:], out_ring[oslot][:])
                    nc.sync.dma_start(
                        out_d[t0:t0 + OB, :, :].rearrange("t p c -> p t c"),
                        ob_sb[:].rearrange("p (t c) -> p t c", t=OB))

            total = NS * (T + 2)
            for tau in range(total + PHASE):
                if tau < total:
                    stage(0, tau // NS, tau % NS)
                t2 = tau - PHASE
                if 0 <= t2 < total:
                    stage(1, t2 // NS, t2 % NS)
    nc.compile()
    return nc


def make_in_maps(inputs):
    x = np.asarray(inputs["x"], np.float32)          # [512, 1024, 20]
    params = prep_params(**{k: v for k, v in inputs.items() if k != "x"})
    in_maps = []
    for c in range(N_CORES):
        xc = x[c * b:(c + 1) * b]                    # [64, T, 20]
        xtc = np.ascontiguousarray(xc.transpose(2, 1, 0))   # [20, T, 64]
        m = {"xt": xtc.astype(NP_CDT),
             "ones": np.ones((1, OB * b), NP_CDT)}
        m.update({k: v.astype(NP_CDT) for k, v in params.items()})
        in_maps.append(m)
    return in_maps


def gather_out(res, B, T):
    out = np.empty((B, T, OUT_DIM), np.float32)
    for c in range(N_CORES):
        out[c * b:(c + 1) * b] = res.results[c]["out"].transpose(2, 0, 1)
    return out


def kernel(**inputs):
    from concourse.bass_utils import run_bass_kernel_spmd

    x = np.asarray(inputs["x"], np.float32)
    B, T, _ = x.shape
    nc = build_nc(T)
    in_maps = make_in_maps(inputs)

    res = run_bass_kernel_spmd(nc, in_maps, core_ids=list(range(N_CORES)))
    return gather_out(res, B, T)


# revision 14
# speedup vs baseline: 1.2416x; 1.0558x over previous
"""Trainium2 Bass kernel for the 3-layer LSTM scan (nn_Net_2095944040841).

Architecture (per core, batch-sharded 512/8 = 64):
  reference: hid = x @ W1.T + b1; 3 chained LSTMCells over T=1024 with the
  original model's quirky state handling (c3 stays 0; cell3 overwrites c2,
  which cell2 reads the next step).

Mapping (v3):
  - Gate rows on partitions packed [i; f] and [g; o]; batch on the free axis.
  - K-stacked stationaries: each (cell, gate-group) is ONE matmul with a
    [128, 128] stationary holding input weights (rows 0:K_in), the bias
    (row 63) and recurrent weights (rows 64:114). The moving state tile per
    group/parity holds [input | bias | h_self] on the matching rows, so a
    tick needs only 6 gate matmuls per group.
  - Both gate groups of all 3 cells share ONE PSUM tile so a single Tanh
    covers every gate per tick.
  - Sigmoids via tanh (weights pre-scaled); states scaled h_hat=2h, s_hat=2c.
  - Skew: at tick k cell1 computes t=k, cell2 t=k-1, cell3 t=k-2. Blocks
    [c1|c2|c3]. Cell3's c-input is 0; cell2's c-input is cell3's same-tick
    i*g (a2 blk3 stays 0 so one stt updates every c).
  - h is written twice by two parallel stts: into self rows (64:128) and into
    the next cells' input rows (0:50) - no serial staging copy.
  - Output matmul batched over OB ticks from an h3 history buffer.
  - Two batch groups emitted 4 pipeline stages out of phase to hide the
    serial chain latency (engines execute queues in order).
  - Small elementwise ops (a2, x/hist copies) run on GPSIMD.
"""

import sys

sys.path.insert(0, "/opt/trn_rl_repo")

import numpy as np

import concourse.bass as bass
import concourse.tile as tile
from concourse import bacc, mybir

HID = 50
IN_DIM = 20
OUT_DIM = 8
T_FULL = 1024
N_CORES = 8

b = 64          # batch per core
G = 2           # interleaved batch groups per core
bg = b // G     # batch per group
TB = 3 * bg     # merged tile width per group; blocks [c1 | c2 | c3]
CH = 64         # x-chunk length in ticks
OB = 8          # output accumulation ticks per DMA
BIAS_ROW = 63   # constant-1.0 row in the moving state tile
NS = 8          # pipeline stages per tick
PHASE = 5       # group-1 lag in stages

F32 = mybir.dt.float32
BF16 = mybir.dt.bfloat16
CDT = BF16
import ml_dtypes
NP_CDT = ml_dtypes.bfloat16

GATES = {"i": slice(0, 50), "f": slice(50, 100), "g": slice(100, 150),
         "o": slice(150, 200)}


def prep_params(W1, b1, Wih1, Whh1, bih1, bhh1, Wih2, Whh2, bih2, bhh2,
                Wih3, Whh3, bih3, bhh3, W2, b2):
    """Host-side weight transformation -> {name: np.float32 array}.

    Stationary per (cell, group) is [128, 128]:
      rows 0:K_in   = input weights (cell1: Wih1@W1 on x; cells2/3: Wih on
                      h_hat_prev, pre-halved)
      row  BIAS_ROW = total bias
      rows 64:114   = recurrent weights on h_hat_self (pre-halved)
    columns: gate A -> 0:50, gate B -> 64:114 (groups "if", "go").
    """
    W1 = np.asarray(W1, np.float32)
    Wc1 = np.asarray(Wih1, np.float32) @ W1            # [200, 20]
    bc1 = (np.asarray(Wih1, np.float32) @ np.asarray(b1, np.float32)
           + np.asarray(bih1, np.float32) + np.asarray(bhh1, np.float32))
    cells = {
        1: (Wc1, np.asarray(Whh1, np.float32), bc1, 1.0),
        2: (np.asarray(Wih2, np.float32), np.asarray(Whh2, np.float32),
            np.asarray(bih2, np.float32) + np.asarray(bhh2, np.float32), 0.5),
        3: (np.asarray(Wih3, np.float32), np.asarray(Whh3, np.float32),
            np.asarray(bih3, np.float32) + np.asarray(bhh3, np.float32), 0.5),
    }
    out = {}
    for c, (Wx, Wh, bias, in_scale) in cells.items():
        gs = {g: (0.5 if g in "ifo" else 1.0) for g in "ifgo"}
        blk = {g: gs[g] * in_scale * Wx[GATES[g]] for g in "ifgo"}   # [50, K]
        blkh = {g: gs[g] * 0.5 * Wh[GATES[g]] for g in "ifgo"}       # [50, 50]
        bb = {g: gs[g] * bias[GATES[g]] for g in "ifgo"}
        Kx = Wx.shape[1]
        for gname, (ga, gb) in (("if", ("i", "f")), ("go", ("g", "o"))):
            w = np.zeros((128, 128), np.float32)
            w[0:Kx, 0:50] = blk[ga].T
            w[0:Kx, 64:114] = blk[gb].T
            w[BIAS_ROW, 0:50] = bb[ga]
            w[BIAS_ROW, 64:114] = bb[gb]
            w[64:114, 0:50] = blkh[ga].T
            w[64:114, 64:114] = blkh[gb].T
            out[f"w{c}_{gname}"] = w
    w2e = np.zeros((51, OUT_DIM), np.float32)
    w2e[0:50, :] = 0.5 * np.asarray(W2, np.float32).T
    w2e[50, :] = np.asarray(b2, np.float32)
    out["w2e"] = w2e
    return out


def build_nc(T=T_FULL):
    """Build the Bass module for one core (SPMD across 8)."""
    nc = bacc.Bacc(None, target_bir_lowering=False)
    BLK = [0, bg, 2 * bg]

    xt = nc.dram_tensor("xt", [IN_DIM, T, b], CDT, kind="ExternalInput")
    wnames = {}
    for c in (1, 2, 3):
        for g in ("if", "go"):
            wnames[f"w{c}_{g}"] = nc.dram_tensor(
                f"w{c}_{g}", [128, 128], CDT, kind="ExternalInput")
    w2e_d = nc.dram_tensor("w2e", [51, OUT_DIM], CDT, kind="ExternalInput")
    ones_d = nc.dram_tensor("ones", [1, OB * b], CDT, kind="ExternalInput")
    out_d = nc.dram_tensor("out", [T, OUT_DIM, b], F32, kind="ExternalOutput")

    n_chunks = (T + CH - 1) // CH
    Tanh = mybir.ActivationFunctionType.Tanh
    ADD, MULT = mybir.AluOpType.add, mybir.AluOpType.mult

    with tile.TileContext(nc) as tc:
        with (
            tc.tile_pool(name="weights", bufs=1) as wp,
            tc.tile_pool(name="state", bufs=1) as sp,
            tc.tile_pool(name="xs", bufs=1) as xp,
            tc.tile_pool(name="work", bufs=3) as wk,
            tc.tile_pool(name="psum", bufs=2, space="PSUM") as pp,
            tc.tile_pool(name="opsum", bufs=1, space="PSUM") as op_pool,
        ):
            wt = {}
            for name, d in wnames.items():
                t = wp.tile(list(d.shape), CDT, name=name, tag=name)
                nc.sync.dma_start(t[:], d[:])
                wt[name] = t
            w2e = wp.tile([128, OUT_DIM], CDT)
            nc.sync.dma_start(w2e[64:115, :], w2e_d[:])

            # moving state tiles: rows 0:50 input (x / h_hat_prev),
            # row 63 = 1.0, rows 64:128 h_hat_self (+junk), per group/parity
            M = [[sp.tile([128, TB], CDT, name=f"M{g}_{i}", tag=f"M{g}_{i}")
                  for i in range(2)] for g in range(G)]
            s_ring = [[sp.tile([128, TB], CDT, name=f"s{g}_{i}", tag=f"s{g}_{i}")
                       for i in range(2)] for g in range(G)]
            a2t = [[sp.tile([128, TB], CDT, name=f"a2_{g}_{i}", tag=f"a2_{g}_{i}")
                    for i in range(2)] for g in range(G)]
            hist = [sp.tile([128, OB * b], CDT, name=f"hist{i}", tag=f"hist{i}")
                    for i in range(2)]
            for g in range(G):
                for i in range(2):
                    nc.vector.memset(M[g][i][0:64, :], 0.0)
                    nc.vector.memset(M[g][i][64:128, :], 0.0)
                    nc.sync.dma_start(M[g][i][BIAS_ROW:BIAS_ROW + 1, :],
                                      ones_d[:, 0:TB])
                    nc.vector.memset(s_ring[g][i][64:128, :], 0.0)
                    nc.vector.memset(a2t[g][i][64:128, 2 * bg:3 * bg], 0.0)
            for i in range(2):
                nc.vector.memset(hist[i][64:128, :], 0.0)
                nc.sync.dma_start(hist[i][114:115, :], ones_d[:])

            xs_ring = [xp.tile([IN_DIM, CH, b], CDT, name=f"xs{i}", tag=f"xs{i}")
                       for i in range(2)]
            nc.sync.dma_start(xs_ring[0][:], xt[:, 0:CH, :])
            # pre-stage x(0) into the tick-0 moving tiles (parity 1)
            for g in range(G):
                nc.gpsimd.tensor_copy(
                    M[g][1][0:IN_DIM, 0:bg],
                    xs_ring[0][:, 0, g * bg:(g + 1) * bg])

            out_ring = [op_pool.tile([OUT_DIM, OB * b], F32,
                                     name=f"ob{i}", tag=f"ob{i}")
                        for i in range(2)]

            # per-(group, parity) work tiles are pooled by tag
            cur = {}   # live tiles per group: P, S, at1, tcx

            def stage(g, k, s):
                p, q = k % 2, (k - 1) % 2
                t1 = min(k, T - 1)
                c_idx = t1 // CH
                if s == 0:
                    if g == 0 and k % CH == 0 and k // CH == c_idx \
                            and c_idx + 1 < n_chunks:
                        nc.sync.dma_start(
                            xs_ring[(c_idx + 1) % 2][:],
                            xt[:, (c_idx + 1) * CH:(c_idx + 2) * CH, :])
                    P = pp.tile([128, 2 * TB], F32, name=f"P{g}", tag=f"P{g}")
                    cur[(g, "P")] = P
                    for (gg, off) in (("if", 0), ("go", TB)):
                        for c in (1, 2, 3):
                            lo = off + BLK[c - 1]
                            nc.tensor.matmul(
                                P[:, lo:lo + bg], wt[f"w{c}_{gg}"][:],
                                M[g][q][:, BLK[c - 1]:BLK[c - 1] + bg],
                                start=True, stop=True)
                elif s == 1:
                    S = wk.tile([128, 2 * TB], CDT, name=f"S{g}", tag=f"S{g}")
                    cur[(g, "S")] = S
                    nc.scalar.activation(S[:], cur[(g, "P")][:], Tanh)
                elif s == 2:
                    S = cur[(g, "S")]
                    at1 = wk.tile([128, TB], CDT, name=f"at1_{g}",
                                  tag=f"at1_{g}")
                    cur[(g, "at1")] = at1
                    nc.vector.scalar_tensor_tensor(
                        at1[64:128, :], S[0:64, 0:TB], 1.0, S[0:64, TB:2 * TB],
                        ADD, MULT)
                elif s == 3:
                    S, at1 = cur[(g, "S")], cur[(g, "at1")]
                    a2 = a2t[g][p]
                    nc.vector.scalar_tensor_tensor(
                        a2[64:128, 0:bg], S[64:128, 0:bg], 1.0,
                        s_ring[g][q][64:128, 0:bg], ADD, MULT)
                    if k == 1:
                        nc.vector.memset(a2[64:128, bg:2 * bg], 0.0)
                    else:
                        nc.vector.scalar_tensor_tensor(
                            a2[64:128, bg:2 * bg], S[64:128, bg:2 * bg], 1.0,
                            at1[64:128, 2 * bg:3 * bg], ADD, MULT)
                elif s == 4:
                    nc.vector.scalar_tensor_tensor(
                        s_ring[g][p][64:128, :], a2t[g][p][64:128, :], 0.5,
                        cur[(g, "at1")][64:128, :], MULT, ADD)
                elif s == 5:
                    tcx = wk.tile([128, TB], CDT, name=f"tc{g}", tag=f"tc{g}")
                    cur[(g, "tcx")] = tcx
                    nc.scalar.activation(tcx[64:128, :],
                                         s_ring[g][p][64:128, :], Tanh,
                                         scale=0.5)
                elif s == 6:
                    S, tcx = cur[(g, "S")], cur[(g, "tcx")]
                    # h_hat into self rows of M[p] (junk rows 114:128 hit
                    # zero stationary rows)
                    nc.vector.scalar_tensor_tensor(
                        M[g][p][64:128, :], S[64:128, TB:2 * TB], 1.0,
                        tcx[64:128, :], ADD, MULT)
                    # h_hat of cells 1,2 into input rows of blocks 2,3
                    nc.vector.scalar_tensor_tensor(
                        M[g][p][0:50, bg:3 * bg],
                        S[64:114, TB:TB + 2 * bg], 1.0,
                        tcx[64:114, 0:2 * bg], ADD, MULT)
                    if k == 0:
                        # cells 2,3 computed garbage steps t<0: zero their
                        # self-h; zero cell3's input h2 (t=-1). Cell2's input
                        # h1(t=0) is real and must survive.
                        nc.vector.memset(M[g][0][64:128, bg:3 * bg], 0.0)
                        nc.vector.memset(M[g][0][0:50, 2 * bg:3 * bg], 0.0)
                    elif k == 1:
                        # cell3 is still at t=-1: zero only its self-h
                        nc.vector.memset(M[g][1][64:128, 2 * bg:3 * bg], 0.0)
                    # x(t=k+1) into input rows of block 1 (next parity tile)
                    tn = min(k + 1, T - 1)
                    nc.gpsimd.tensor_copy(
                        M[g][p][0:IN_DIM, 0:bg],
                        xs_ring[(tn // CH) % 2][:, tn % CH,
                                                g * bg:(g + 1) * bg])
                    # h3_hat into the history buffer for the batched out-mm
                    if k >= 2:
                        t3 = k - 2
                        nc.gpsimd.tensor_copy(
                            hist[(t3 // OB) % 2][64:114,
                                                 (t3 % OB) * b + g * bg:
                                                 (t3 % OB) * b + (g + 1) * bg],
                            M[g][p][64:114, 2 * bg:3 * bg])
                elif s == 7:
                    # trailing group emits the batched output for both groups
                    if g != G - 1 or k < 2:
                        return
                    t3 = k - 2
                    if t3 % OB != OB - 1:
                        return
                    oslot = (t3 // OB) % 2
                    t0 = t3 - OB + 1
                    nc.tensor.matmul(out_ring[oslot][:],
                                     w2e[64:115, :], hist[oslot][64:115, :],
                                     start=True, stop=True)
                    ob_sb = wk.tile([OUT_DIM, OB * b], F32, name="ob_sb",
                                    tag="ob_sb")
                    nc.scalar.copy(ob_sb[:], out_ring[oslot][:])
                    nc.sync.dma_start(
                        out_d[t0:t0 + OB, :, :].rearrange("t p c -> p t c"),
                        ob_sb[:].rearrange("p (t c) -> p t c", t=OB))

            total = NS * (T + 2)
            for tau in range(total + PHASE):
                if tau < total:
                    stage(0, tau // NS, tau % NS)
                t2 = tau - PHASE
                if 0 <= t2 < total:
                    stage(1, t2 // NS, t2 % NS)
    nc.compile()
    return nc


def make_in_maps(inputs):
    x = np.asarray(inputs["x"], np.float32)          # [512, 1024, 20]
    params = prep_params(**{k: v for k, v in inputs.items() if k != "x"})
    in_maps = []
    for c in range(N_CORES):
        xc = x[c * b:(c + 1) * b]                    # [64, T, 20]
        xtc = np.ascontiguousarray(xc.transpose(2, 1, 0))   # [20, T, 64]
        m = {"xt": xtc.astype(NP_CDT),
             "ones": np.ones((1, OB * b), NP_CDT)}
        m.update({k: v.astype(NP_CDT) for k, v in params.items()})
        in_maps.append(m)
    return in_maps


def gather_out(res, B, T):
    out = np.empty((B, T, OUT_DIM), np.float32)
    for c in range(N_CORES):
        out[c * b:(c + 1) * b] = res.results[c]["out"].transpose(2, 0, 1)
    return out


def kernel(**inputs):
    from concourse.bass_utils import run_bass_kernel_spmd

    x = np.asarray(inputs["x"], np.float32)
    B, T, _ = x.shape
    nc = build_nc(T)
    in_maps = make_in_maps(inputs)

    res = run_bass_kernel_spmd(nc, in_maps, core_ids=list(range(N_CORES)))
    return gather_out(res, B, T)
